# revision 29
# baseline (speedup 1.0000x reference)
"""Trainium2 Bass kernel for CausalSelfAttention with lightning (linear)
attention + LRPE, sharded over 8 NeuronCores.

Model (reference):
    qkv = x @ w_qkv.T ; split q,k,v ; per-head LRPE on q,k (dims e -> 2e)
    chunked linear attention with per-head exponential decay
    y = attn output ; out = y @ w_proj.T

Shapes: x (4, 2048, 2048), w_qkv (6144, 2048), w_proj (2048, 2048),
theta (16, 1, 128). 16 heads, head dim 128.

Sharding: 8 cores = (batch 4) x (head-group 2, 8 heads each). Each core
computes a partial output (2048, 2048) = y_part @ w_proj[:, cols].T in
fp16; host upcasts and sums the two partials per batch.

v2: fully fused pipeline, all pools co-resident so the Tile scheduler can
overlap everything. Emission: v projection (4 column-quarters), then per
head h a "self-zipped" slot interleaving the head's 8 qk-projection
sub-tiles with its 8 lightning-attention chunks, then the output
projection (4 column-quarters). Engine split:
  DVE:  LRPE multiplies + q-decay scaling (all fp16 -> 2x mode)
  Pool: score-mask multiply + decayed-state update
  Act:  k-decay scaling of v, y chunk copy (PSUM->SBUF)
  DMA:  k transposes (XBAR dma transpose), all streaming (prefetched a
        slot or more ahead; stream pools sized so no ring-slot aliases a
        still-live tile, which would head-of-line block a DMA queue)
  PE:   matmuls only (fp16 / fp32r, moving dim 256-512)
qk T-layout tiles go straight from PSUM into SBUF ring slots (no DRAM
round trip); v and y spill to DRAM and stream back.
"""
import contextlib
import math

import numpy as np

import concourse.tile as tile
from concourse import bacc, mybir
from concourse import bass_utils

F32 = mybir.dt.float32
F32R = mybir.dt.float32r
F16 = mybir.dt.float16

P = 128
DIM = 2048
HEADS = 16
B = 4
T = 2048
E = DIM // HEADS          # 128
HPC = HEADS // 2          # heads per core = 8
CHUNK = 256               # chunk size (exact identity holds for any size)
NCH = T // CHUNK          # 8 chunks
KC = DIM // P             # 16 contraction chunks of 128
NT = T // 512             # 4 token tiles of 512
QK_DIMS = 2 * HPC * E     # 2048 (pair-interleaved: h*256+[q128|k128])
YD = HPC * E              # 1024 y dims per core

_NC_CACHE = None

# optional emission-site attribution for sim debugging:
# maps instruction name -> label of the emission region
_LABEL = [None]
_LABEL_MAP = {}


def _set_label(lbl):
    _LABEL[0] = lbl


def _build_nc(loop_n: int = 1, phases: str = "123"):
    """Build the (SPMD-identical) Bass program for one core.

    loop_n > 1 wraps the body in a hardware loop (benchmarking only).
    phases is accepted for legacy tooling and ignored (always full)."""
    nc = bacc.Bacc("TRN2", target_bir_lowering=False, debug=False,
                   enable_asserts=False, num_devices=8)
    _orig_next = nc.get_next_instruction_name

    def _tracked():
        name = _orig_next()
        if _LABEL[0] is not None:
            _LABEL_MAP[name] = _LABEL[0]
        return name

    nc.get_next_instruction_name = _tracked

    xt_d = nc.dram_tensor("xt", (DIM, T), F16, kind="ExternalInput")        # x_b.T
    wt_d = nc.dram_tensor("wt", (DIM, QK_DIMS), F16, kind="ExternalInput")  # W_qk.T pairs
    wv_d = nc.dram_tensor("wv", (DIM, YD), F16, kind="ExternalInput")       # W_v.T
    wp_d = nc.dram_tensor("wp", (YD, DIM), F16, kind="ExternalInput")       # w_proj[:, cols].T
    cos_d = nc.dram_tensor("costab", (YD, T), F16, kind="ExternalInput")
    sin_d = nc.dram_tensor("sintab", (YD, T), F16, kind="ExternalInput")
    mask_d = nc.dram_tensor("maskt", (HPC, 2, P, CHUNK), F16, kind="ExternalInput")
    qdec_d = nc.dram_tensor("qdec", (HPC, P, CHUNK), F16, kind="ExternalInput")
    kdec_d = nc.dram_tensor("kdec", (HPC, 2, P), F32, kind="ExternalInput")
    lamc_d = nc.dram_tensor("lamc", (HPC, P), F32, kind="ExternalInput")
    out_d = nc.dram_tensor("out", (T, DIM), F16, kind="ExternalOutput")

    with tile.TileContext(nc) as tc:
        with (
            tc.tile_pool(name="const", bufs=1) as constp,
            tc.tile_pool(name="dram", bufs=1, space="DRAM") as dram,
            tc.tile_pool(name="p1x", bufs=1) as p1x,
            tc.tile_pool(name="wq", bufs=2) as wq,
            tc.tile_pool(name="qkp", bufs=3) as qkp,
            tc.tile_pool(name="vstr", bufs=3) as vstr,
            tc.tile_pool(name="csstr", bufs=3) as csstr,
            tc.tile_pool(name="p2c", bufs=2) as p2c,
            tc.tile_pool(name="stp", bufs=2) as stp,
            tc.tile_pool(name="p1so", bufs=3) as p1so,
            tc.tile_pool(name="p3so", bufs=3) as p3so,
            tc.tile_pool(name="p3y", bufs=3) as p3y,
            tc.tile_pool(name="mmps", bufs=2, space="PSUM") as mmps,
            tc.tile_pool(name="p2ps", bufs=1, space="PSUM") as p2ps,
        ):
            vnds = [dram.tile([P, KC, E], F16, name=f"vnd{h}", tag=f"vnd{h}")
                    for h in range(HPC)]
            ylTs = [dram.tile([P, KC // 2, HPC, P], F16, name=f"ylT{x}",
                              tag=f"ylT{x}") for x in range(2)]

            env = dict(locals())
            loop_cm = tc.For_i(0, loop_n, 1) if loop_n > 1 else contextlib.nullcontext()
            with loop_cm:
                _emit(nc, tc, env)

    nc.compile()
    return nc


def _emit(nc, tc, env):
    mult = mybir.AluOpType.mult
    add = mybir.AluOpType.add
    COPY = mybir.ActivationFunctionType.Copy
    xt_d = env["xt_d"]; wt_d = env["wt_d"]; wv_d = env["wv_d"]; wp_d = env["wp_d"]
    cos_d = env["cos_d"]; sin_d = env["sin_d"]
    mask_d = env["mask_d"]; qdec_d = env["qdec_d"]
    kdec_d = env["kdec_d"]; lamc_d = env["lamc_d"]; out_d = env["out_d"]
    constp = env["constp"]; p1x = env["p1x"]; wq = env["wq"]
    qkp = env["qkp"]; vstr = env["vstr"]; csstr = env["csstr"]
    p2c = env["p2c"]; stp = env["stp"]; p1so = env["p1so"]; p3so = env["p3so"]
    p3y = env["p3y"]; mmps = env["mmps"]; p2ps = env["p2ps"]
    vnds = env["vnds"]; ylTs = env["ylTs"]

    # copies out of PSUM alternate between DVE and Act deterministically
    _cp_state = [0]

    def copy_out(dst, src):
        if _cp_state[0] % 2 == 0:
            nc.vector.tensor_copy(out=dst, in_=src)
        else:
            nc.scalar.copy(dst, src)
        _cp_state[0] += 1

    wv_src = wv_d.ap().rearrange("(kc p) m -> p kc m", p=P)
    wt_src = wt_d.ap().rearrange("(kc p) m -> p kc m", p=P)
    wp_src = wp_d.ap().rearrange("(kc p) n -> p kc n", p=P)

    def wtile(a, b):
        """Ring slot from the shared weights pool as [P, a, b] fp16
        (a*b == 4096)."""
        t = wq.tile([P, 4096], F16, tag="w", name="wslot")
        return t.rearrange("p (a b) -> p a b", a=a)

    # ---- first x chunk + v weight quarter ahead so the PE starts ASAP ----
    xt_src = xt_d.ap().rearrange("(kc p) t -> p kc t", p=P)
    xts = [p1x.tile([P, T], F16, name="xt0", tag="xt0")]
    nc.sync.dma_start(xts[0], xt_src[:, 0])
    wv_t0 = wtile(KC, 256)
    for pc in range(4):
        nc.sync.dma_start(wv_t0[:, 4 * pc:4 * pc + 4],
                          wv_src[:, 4 * pc:4 * pc + 4, 0:256])

    # ---- resident x (T-layout, fp16) ----
    for kc in range(1, KC):
        xts.append(p1x.tile([P, T], F16, name=f"xt{kc}", tag=f"xt{kc}"))
        nc.sync.dma_start(xts[kc], xt_src[:, kc])

    # ---- small constant tables (needed from head slot 0 on) ----
    ident16 = constp.tile([P, P], F16, tag="ident")
    from concourse.masks import make_identity
    make_identity(nc, ident16)
    kdec_t = constp.tile([P, HPC, 2], F32, tag="kdec")
    nc.sync.dma_start(kdec_t, kdec_d.ap().rearrange("h j p -> p h j"))
    lamc_t = constp.tile([P, HPC], F32, tag="lamc")
    nc.sync.dma_start(lamc_t, lamc_d.ap().rearrange("h p -> p h"))
    mask_t = constp.tile([P, HPC, 2, CHUNK], F16, tag="mask")
    nc.sync.dma_start(mask_t, mask_d.ap().rearrange("h j p c -> p h j c"))
    qdec_t = constp.tile([P, HPC, CHUNK], F16, tag="qdec")
    nc.sync.dma_start(qdec_t, qdec_d.ap().rearrange("h p c -> p h c"))

    # ================= v projection: 4 column-quarters =================
    def prod_v(q, wv_t=None):
        _set_label(f"v{q}")
        if wv_t is None:
            wv_t = wtile(KC, 256)
            nc.sync.dma_start(wv_t, wv_src[:, :, q * 256:(q + 1) * 256])
        for mt in range(T // P):
            psf = mmps.tile([P, 512], F32, tag="mm")
            ps = psf[:, 0:256]
            for kc in range(KC):
                nc.tensor.matmul(ps, xts[kc][:, mt * P:(mt + 1) * P],
                                 wv_t[:, kc],
                                 start=(kc == 0), stop=(kc == KC - 1))
            so = p1so.tile([P, 256], F16, tag="so")
            copy_out(so, ps)
            nc.sync.dma_start(vnds[2 * q][:, mt], so[:, 0:E])
            nc.sync.dma_start(vnds[2 * q + 1][:, mt], so[:, E:2 * E])

    # ================= per-head streaming prefetch =================
    pf_w, pf_cs, pf_v = {}, {}, {}

    def prefetch_w(h):
        _set_label(f"pfw{h}")
        wqk = wtile(KC, 256)
        nc.sync.dma_start(wqk, wt_src[:, :, h * 256:(h + 1) * 256])
        pf_w[h] = wqk

    def prefetch_cs(h):
        _set_label(f"pfcs{h}")
        cost = csstr.tile([P, T], F16, tag="cos")
        sint = csstr.tile([P, T], F16, tag="sin")
        r = h * P
        nc.sync.dma_start(cost, cos_d.ap()[r:r + P])
        nc.sync.dma_start(sint, sin_d.ap()[r:r + P])
        pf_cs[h] = (cost, sint)

    def prefetch_v(h):
        _set_label(f"pfv{h}")
        vna = vstr.tile([P, KC, E], F16, tag="vna")
        nc.sync.dma_start(vna, vnds[h])
        pf_v[h] = vna

    # ================= qk producer (8 substeps: q0 k0 q1 k1 ...) =========
    def prod_qk(h):
        wqk = pf_w[h]
        qs = qkp.tile([P, T], F16, tag="qs")
        ks = qkp.tile([P, T], F16, tag="ks")
        qk_slots[h] = (qs, ks)
        pending = None
        for n in range(NT):
            for which, dst in ((0, qs), (1, ks)):
                _set_label(f"s{h}.mm{n}{'qk'[which]}")
                ps = mmps.tile([P, 512], F32, tag="mm")
                for kc in range(KC):
                    nc.tensor.matmul(ps, wqk[:, kc, which * P:(which + 1) * P],
                                     xts[kc][:, n * 512:(n + 1) * 512],
                                     start=(kc == 0), stop=(kc == KC - 1))
                # the copy-out is emitted AFTER the driver's attention pulls
                # for this substep: a copy whose PSUM isn't complete yet
                # would otherwise head-of-line block ready LRPE work queued
                # behind it on the same engine.
                yield
                _set_label(f"s{h}.cp{n}{'qk'[which]}")
                copy_out(dst[:, n * 512:(n + 1) * 512], ps)

    qk_slots = {}

    # ================= attention consumer =================
    def attn(h):
        """Yields 8 times; yield i emits B(c_{i-1}) then A(c_i), with B(c7)
        emitted on the final drain. A: LRPE + scores + mask + transposes;
        B: output + state update. The B-before-A pairing keeps one full
        producer substep of pipeline lead between a chunk's A-phase
        (cross-engine producers) and its B-phase (PE consumers)."""
        qs, ks = qk_slots[h]
        cost, sint = pf_cs[h]
        vna = pf_v[h]
        lam_col = lamc_t[:, h:h + 1]

        st_prev = [None, None]
        ab = [None] * NCH

        def phase_a(i):
            _set_label(f"a{h}.A{i}")
            sl = slice(i * CHUNK, (i + 1) * CHUNK)
            qlc = p2c.tile([P, CHUNK], F16, tag="qlc")
            qls = p2c.tile([P, CHUNK], F16, tag="qls")
            klc = p2c.tile([P, CHUNK], F16, tag="klc")
            kls = p2c.tile([P, CHUNK], F16, tag="kls")
            nc.vector.tensor_tensor(klc, ks[:, sl], cost[:, sl], mult)
            nc.vector.tensor_tensor(qlc, qs[:, sl], cost[:, sl], mult)
            nc.vector.tensor_tensor(kls, ks[:, sl], sint[:, sl], mult)
            nc.vector.tensor_tensor(qls, qs[:, sl], sint[:, sl], mult)
            # scoresT (both j-halves in one bank) then mask multiply on Pool
            sco = p2ps.tile([P, 2, CHUNK], F32, tag="sco", bufs=2)
            for jh in range(2):
                jsl = slice(jh * P, (jh + 1) * P)
                nc.tensor.matmul(sco[:, jh], klc[:, jsl], qlc,
                                 start=True, stop=False)
                nc.tensor.matmul(sco[:, jh], kls[:, jsl], qls,
                                 start=False, stop=True)
            smk = p2c.tile([P, 2, CHUNK], F16, tag="smk")
            nc.vector.tensor_tensor(smk, sco, mask_t[:, h], mult)
            # k natural layout via PE transpose (in the PE stream -- no
            # cross-engine launch chain) + k-decay fold on the Act copy out
            # of PSUM. Only consumed by the B-phase state update.
            knat = None
            if i < NCH - 1:
                knat = p2c.tile([P, 2, 2 * E], F16, tag="knat")
                ptr = p2ps.tile([P, 2, 2 * E], F16, tag="ptr", bufs=2)
                for half in range(2):
                    hsl = slice(half * P, (half + 1) * P)
                    kd = kdec_t[:, h, half:half + 1]
                    nc.tensor.transpose(ptr[:, half, 0:E], klc[:, hsl], ident16)
                    nc.tensor.transpose(ptr[:, half, E:2 * E], kls[:, hsl], ident16)
                    nc.scalar.activation(knat[:, half, 0:E], ptr[:, half, 0:E],
                                         COPY, bias=0.0, scale=kd)
                    nc.scalar.activation(knat[:, half, E:2 * E],
                                         ptr[:, half, E:2 * E],
                                         COPY, bias=0.0, scale=kd)
            ab[i] = (qlc, qls, knat, None, smk)

        def phase_b(i):
            _set_label(f"a{h}.B{i}")
            nonlocal st_prev
            sl = slice(i * CHUNK, (i + 1) * CHUNK)
            qlc, qls, knat, vnak, smk = ab[i]
            ops = p2ps.tile([E, CHUNK], F32, tag="ops", bufs=1)
            nc.tensor.matmul(ops, vna[:, 2 * i], smk[:, 0],
                             start=True, stop=False)
            nc.tensor.matmul(ops, vna[:, 2 * i + 1], smk[:, 1],
                             start=False, stop=(i == 0))
            if i > 0:
                qsc = p2c.tile([P, CHUNK], F16, tag="qsc")
                qss = p2c.tile([P, CHUNK], F16, tag="qss")
                nc.gpsimd.tensor_tensor(qsc, qlc, qdec_t[:, h], mult)
                nc.gpsimd.tensor_tensor(qss, qls, qdec_t[:, h], mult)
                nc.tensor.matmul(ops, st_prev[0], qsc, start=False, stop=False)
                nc.tensor.matmul(ops, st_prev[1], qss, start=False, stop=True)
            yst = p2c.tile([E, CHUNK], F16, tag="yst")
            nc.scalar.copy(yst, ops)
            half, io = divmod(i, NCH // 2)
            nc.scalar.dma_start(ylTs[half][:, 2 * io:2 * io + 2, h], yst)
            if i < NCH - 1:
                sad = p2ps.tile([P, 2, E], F32, tag="sad", bufs=1)
                nc.tensor.matmul(sad[:, 0], knat[:, 0, 0:E], vna[:, 2 * i],
                                 start=True, stop=False)
                nc.tensor.matmul(sad[:, 0], knat[:, 1, 0:E], vna[:, 2 * i + 1],
                                 start=False, stop=True)
                nc.tensor.matmul(sad[:, 1], knat[:, 0, E:2 * E], vna[:, 2 * i],
                                 start=True, stop=False)
                nc.tensor.matmul(sad[:, 1], knat[:, 1, E:2 * E],
                                 vna[:, 2 * i + 1],
                                 start=False, stop=True)
                st0 = stp.tile([P, E], F16, tag="st0")
                st1 = stp.tile([P, E], F16, tag="st1")
                if i == 0:
                    nc.vector.tensor_copy(out=st0, in_=sad[:, 0])
                    nc.vector.tensor_copy(out=st1, in_=sad[:, 1])
                else:
                    nc.vector.scalar_tensor_tensor(
                        out=st0, in0=st_prev[0], scalar=lam_col,
                        in1=sad[:, 0], op0=mult, op1=add)
                    nc.vector.scalar_tensor_tensor(
                        out=st1, in0=st_prev[1], scalar=lam_col,
                        in1=sad[:, 1], op0=mult, op1=add)
                st_prev = [st0, st1]

        for i in range(NCH):
            if i > 0:
                phase_b(i - 1)
            phase_a(i)
            yield
        phase_b(NCH - 1)

    # ================= drive =================
    prod_v(0, wv_t0)
    for q in range(1, 4):
        prod_v(q)
    prefetch_v(0)
    prefetch_v(1)
    prefetch_cs(0)
    prefetch_cs(1)
    prefetch_w(0)     # ring slot of vq2 -- free once vq2's matmuls retire

    # Self-zipped pipeline with a floating consumer queue: after each
    # producer substep, pull one [B(c_{i-1}), A(c_i)] attention step from
    # the oldest unfinished head. Heads have 8 steps and slots 8 substeps,
    # so in steady state head h drains across the second half of slot h and
    # the first half of slot h+1 -- every phase keeps >= 1 substep of
    # pipeline lead for its copy->LRPE->transpose chain and there is no
    # per-head tail bunching.
    consq = []

    def pull(n):
        while n > 0 and consq:
            try:
                next(consq[0])
                n -= 1
            except StopIteration:
                consq.pop(0)

    for h in range(HPC):
        prod = prod_qk(h)
        for j in range(8):
            next(prod)
            if j == 1 and h + 1 < HPC:
                prefetch_w(h + 1)
            if j == 2:
                consq.append(attn(h))
            if j == 3 and h + 2 < HPC:
                prefetch_cs(h + 2)
                prefetch_v(h + 2)
            pull(1)
        for _ in prod:
            pass
    # wp into the (now dead) xts tiles: xts[kc] <- w_proj rows kc*128..+128.
    # The WAR on the last qk pair's reads orders these after pair 7.
    _set_label("p3.wp")
    for kc in range(HPC):
        nc.sync.dma_start(xts[kc], wp_src[:, kc])
    while consq:
        pull(1)

    # ================= output projection (m-outer, yl read once) =========
    for m in range(T // P):
        _set_label(f"p3.m{m}")
        yl = p3y.tile([P, HPC, P], F16, tag="yl")
        nc.sync.dma_start(yl, ylTs[m // 8][:, m % 8])
        for n in range(NT):
            ps = mmps.tile([P, 512], F32, tag="mm")
            for kc in range(HPC):
                nc.tensor.matmul(ps, yl[:, kc],
                                 xts[kc][:, n * 512:(n + 1) * 512],
                                 start=(kc == 0), stop=(kc == HPC - 1))
            so = p3so.tile([P, 512], F16, tag="p3o")
            copy_out(so, ps)
            nc.sync.dma_start(
                out_d.ap()[m * P:(m + 1) * P, n * 512:(n + 1) * 512], so)


def _get_nc():
    global _NC_CACHE
    if _NC_CACHE is None:
        _NC_CACHE = _build_nc()
    return _NC_CACHE


def _slopes(h):
    start = 2.0 ** (-(2.0 ** -(math.log2(h) - 3)))
    return np.array([start ** (i + 1) for i in range(h)], dtype=np.float64)


def _prepare_in_maps(x, w_qkv, w_proj, theta):
    slopes = _slopes(HEADS)
    t = np.arange(T, dtype=np.float64)
    idx = np.arange(CHUNK, dtype=np.float64)

    in_maps = []
    for core in range(8):
        b, g = divmod(core, 2)
        heads = np.arange(g * HPC, (g + 1) * HPC)

        xt = np.ascontiguousarray(x[b].T).astype(np.float16)

        # columns pair-interleaved: head h -> [q rows | k rows]
        qk_rows = np.concatenate([
            np.concatenate([
                np.arange(g * YD + hh * E, g * YD + (hh + 1) * E),
                np.arange(DIM + g * YD + hh * E, DIM + g * YD + (hh + 1) * E),
            ]) for hh in range(HPC)
        ])
        wt = np.ascontiguousarray(w_qkv[qk_rows].T).astype(np.float16)
        v_rows = np.arange(2 * DIM + g * YD, 2 * DIM + (g + 1) * YD)
        wv = np.ascontiguousarray(w_qkv[v_rows].T).astype(np.float16)

        wp = np.ascontiguousarray(w_proj[:, g * YD:(g + 1) * YD].T).astype(np.float16)

        th = theta.reshape(HEADS, E)[heads].astype(np.float64)  # (8, 128)
        ang = th[:, :, None] * t[None, None, :]                 # (8, 128, T)
        costab = np.cos(ang).astype(np.float16).reshape(YD, T)
        sintab = np.sin(ang).astype(np.float16).reshape(YD, T)

        s = slopes[heads]                                       # (8,)
        diff = idx[:, None] - idx[None, :]                      # (i, j)
        maskt = np.where(
            diff[None] >= 0, np.exp(-s[:, None, None] * diff[None]), 0.0
        )                                                       # (8, i, j) = diag_decay
        maskt = np.ascontiguousarray(
            maskt.transpose(0, 2, 1).reshape(HPC, 2, P, CHUNK)).astype(np.float16)
        qdec = np.exp(-s[:, None] * (idx + 1.0)[None]).astype(np.float16)  # (8, 256)
        qdec = np.broadcast_to(qdec[:, None, :], (HPC, P, CHUNK)).copy()
        kdec = np.exp(-s[:, None] * (CHUNK - 1.0 - idx)[None]).astype(np.float32)
        kdec = np.ascontiguousarray(kdec.reshape(HPC, 2, P))
        lamc = np.exp(-s * CHUNK).astype(np.float32)            # (8,)
        lamc = np.broadcast_to(lamc[:, None], (HPC, P)).copy()

        in_maps.append({
            "xt": xt, "wt": wt, "wv": wv, "wp": wp,
            "costab": costab, "sintab": sintab,
            "maskt": maskt, "qdec": qdec, "kdec": kdec, "lamc": lamc,
        })
    return in_maps


def kernel(x, w_qkv, w_proj, theta):
    x = np.asarray(x)
    w_qkv = np.asarray(w_qkv)
    w_proj = np.asarray(w_proj)
    theta = np.asarray(theta)

    nc = _get_nc()
    in_maps = _prepare_in_maps(x, w_qkv, w_proj, theta)
    res = bass_utils.run_bass_kernel_spmd(nc, in_maps, core_ids=list(range(8)))

    out = np.empty((B, T, DIM), dtype=np.float32)
    for b in range(B):
        out[b] = (res.results[2 * b]["out"].astype(np.float32)
                  + res.results[2 * b + 1]["out"].astype(np.float32))
    return out


# revision 30
# speedup vs baseline: 1.0148x; 1.0148x over previous
"""Trainium2 Bass kernel for CausalSelfAttention with lightning (linear)
attention + LRPE, sharded over 8 NeuronCores.

Model (reference):
    qkv = x @ w_qkv.T ; split q,k,v ; per-head LRPE on q,k (dims e -> 2e)
    chunked linear attention with per-head exponential decay
    y = attn output ; out = y @ w_proj.T

Shapes: x (4, 2048, 2048), w_qkv (6144, 2048), w_proj (2048, 2048),
theta (16, 1, 128). 16 heads, head dim 128.

Sharding: 8 cores = (batch 4) x (head-group 2, 8 heads each). Each core
computes a partial output (2048, 2048) = y_part @ w_proj[:, cols].T in
fp16; host upcasts and sums the two partials per batch.

v2: fully fused pipeline, all pools co-resident so the Tile scheduler can
overlap everything. Emission: v projection (4 column-quarters), then per
head h a "self-zipped" slot interleaving the head's 8 qk-projection
sub-tiles with its 8 lightning-attention chunks, then the output
projection (4 column-quarters). Engine split:
  DVE:  LRPE multiplies + q-decay scaling (all fp16 -> 2x mode)
  Pool: score-mask multiply + decayed-state update
  Act:  k-decay scaling of v, y chunk copy (PSUM->SBUF)
  DMA:  k transposes (XBAR dma transpose), all streaming (prefetched a
        slot or more ahead; stream pools sized so no ring-slot aliases a
        still-live tile, which would head-of-line block a DMA queue)
  PE:   matmuls only (fp16 / fp32r, moving dim 256-512)
qk T-layout tiles go straight from PSUM into SBUF ring slots (no DRAM
round trip); v and y spill to DRAM and stream back.
"""
import contextlib
import math

import numpy as np

import concourse.tile as tile
from concourse import bacc, mybir
from concourse import bass_utils

F32 = mybir.dt.float32
F32R = mybir.dt.float32r
F16 = mybir.dt.float16

P = 128
DIM = 2048
HEADS = 16
B = 4
T = 2048
E = DIM // HEADS          # 128
HPC = HEADS // 2          # heads per core = 8
CHUNK = 256               # chunk size (exact identity holds for any size)
NCH = T // CHUNK          # 8 chunks
KC = DIM // P             # 16 contraction chunks of 128
NT = T // 512             # 4 token tiles of 512
QK_DIMS = 2 * HPC * E     # 2048 (pair-interleaved: h*256+[q128|k128])
YD = HPC * E              # 1024 y dims per core

_NC_CACHE = None

# optional emission-site attribution for sim debugging:
# maps instruction name -> label of the emission region
_LABEL = [None]
_LABEL_MAP = {}


def _set_label(lbl):
    _LABEL[0] = lbl


def _build_nc(loop_n: int = 1, phases: str = "123"):
    """Build the (SPMD-identical) Bass program for one core.

    loop_n > 1 wraps the body in a hardware loop (benchmarking only).
    phases is accepted for legacy tooling and ignored (always full)."""
    nc = bacc.Bacc("TRN2", target_bir_lowering=False, debug=False,
                   enable_asserts=False, num_devices=8)
    _orig_next = nc.get_next_instruction_name

    def _tracked():
        name = _orig_next()
        if _LABEL[0] is not None:
            _LABEL_MAP[name] = _LABEL[0]
        return name

    nc.get_next_instruction_name = _tracked

    xt_d = nc.dram_tensor("xt", (DIM, T), F16, kind="ExternalInput")        # x_b.T
    wt_d = nc.dram_tensor("wt", (DIM, QK_DIMS), F16, kind="ExternalInput")  # W_qk.T pairs
    wv_d = nc.dram_tensor("wv", (DIM, YD), F16, kind="ExternalInput")       # W_v.T
    wp_d = nc.dram_tensor("wp", (YD, DIM), F16, kind="ExternalInput")       # w_proj[:, cols].T
    cos_d = nc.dram_tensor("costab", (YD, T), F16, kind="ExternalInput")
    sin_d = nc.dram_tensor("sintab", (YD, T), F16, kind="ExternalInput")
    mask_d = nc.dram_tensor("maskt", (HPC, 2, P, CHUNK), F16, kind="ExternalInput")
    qdec_d = nc.dram_tensor("qdec", (HPC, P, CHUNK), F16, kind="ExternalInput")
    kdec_d = nc.dram_tensor("kdec", (HPC, 2, P), F32, kind="ExternalInput")
    lamc_d = nc.dram_tensor("lamc", (HPC, P), F32, kind="ExternalInput")
    out_d = nc.dram_tensor("out", (T, DIM), F16, kind="ExternalOutput")

    with tile.TileContext(nc) as tc:
        with (
            tc.tile_pool(name="const", bufs=1) as constp,
            tc.tile_pool(name="dram", bufs=1, space="DRAM") as dram,
            tc.tile_pool(name="p1x", bufs=1) as p1x,
            tc.tile_pool(name="wq", bufs=2) as wq,
            tc.tile_pool(name="qkp", bufs=3) as qkp,
            tc.tile_pool(name="vstr", bufs=3) as vstr,
            tc.tile_pool(name="csstr", bufs=3) as csstr,
            tc.tile_pool(name="p2c", bufs=2) as p2c,
            tc.tile_pool(name="stp", bufs=2) as stp,
            tc.tile_pool(name="p1so", bufs=3) as p1so,
            tc.tile_pool(name="p3so", bufs=3) as p3so,
            tc.tile_pool(name="p3y", bufs=3) as p3y,
            tc.tile_pool(name="mmps", bufs=2, space="PSUM") as mmps,
            tc.tile_pool(name="p2ps", bufs=1, space="PSUM") as p2ps,
        ):
            vnds = [dram.tile([P, KC, E], F16, name=f"vnd{h}", tag=f"vnd{h}")
                    for h in range(HPC)]
            ylTs = [dram.tile([P, KC // 2, HPC, P], F16, name=f"ylT{x}",
                              tag=f"ylT{x}") for x in range(2)]

            env = dict(locals())
            loop_cm = tc.For_i(0, loop_n, 1) if loop_n > 1 else contextlib.nullcontext()
            with loop_cm:
                _emit(nc, tc, env)

    nc.compile()
    return nc


def _emit(nc, tc, env):
    mult = mybir.AluOpType.mult
    add = mybir.AluOpType.add
    COPY = mybir.ActivationFunctionType.Copy
    xt_d = env["xt_d"]; wt_d = env["wt_d"]; wv_d = env["wv_d"]; wp_d = env["wp_d"]
    cos_d = env["cos_d"]; sin_d = env["sin_d"]
    mask_d = env["mask_d"]; qdec_d = env["qdec_d"]
    kdec_d = env["kdec_d"]; lamc_d = env["lamc_d"]; out_d = env["out_d"]
    constp = env["constp"]; p1x = env["p1x"]; wq = env["wq"]
    qkp = env["qkp"]; vstr = env["vstr"]; csstr = env["csstr"]
    p2c = env["p2c"]; stp = env["stp"]; p1so = env["p1so"]; p3so = env["p3so"]
    p3y = env["p3y"]; mmps = env["mmps"]; p2ps = env["p2ps"]
    vnds = env["vnds"]; ylTs = env["ylTs"]

    # copies out of PSUM alternate between DVE and Act deterministically
    _cp_state = [0]

    def copy_out(dst, src):
        if _cp_state[0] % 2 == 0:
            nc.vector.tensor_copy(out=dst, in_=src)
        else:
            nc.scalar.copy(dst, src)
        _cp_state[0] += 1

    wv_src = wv_d.ap().rearrange("(kc p) m -> p kc m", p=P)
    wt_src = wt_d.ap().rearrange("(kc p) m -> p kc m", p=P)
    wp_src = wp_d.ap().rearrange("(kc p) n -> p kc n", p=P)

    def wtile(a, b):
        """Ring slot from the shared weights pool as [P, a, b] fp16
        (a*b == 4096)."""
        t = wq.tile([P, 4096], F16, tag="w", name="wslot")
        return t.rearrange("p (a b) -> p a b", a=a)

    # ---- first x chunk + v weight quarter ahead so the PE starts ASAP ----
    xt_src = xt_d.ap().rearrange("(kc p) t -> p kc t", p=P)
    xts = [p1x.tile([P, T], F16, name="xt0", tag="xt0")]
    nc.sync.dma_start(xts[0], xt_src[:, 0])
    wv_t0 = wtile(KC, 256)
    for pc in range(4):
        nc.sync.dma_start(wv_t0[:, 4 * pc:4 * pc + 4],
                          wv_src[:, 4 * pc:4 * pc + 4, 0:256])

    # ---- resident x (T-layout, fp16) ----
    for kc in range(1, KC):
        xts.append(p1x.tile([P, T], F16, name=f"xt{kc}", tag=f"xt{kc}"))
        nc.sync.dma_start(xts[kc], xt_src[:, kc])

    # ---- small constant tables (needed from head slot 0 on) ----
    ident16 = constp.tile([P, P], F16, tag="ident")
    from concourse.masks import make_identity
    make_identity(nc, ident16)
    kdec_t = constp.tile([P, HPC, 2], F32, tag="kdec")
    nc.sync.dma_start(kdec_t, kdec_d.ap().rearrange("h j p -> p h j"))
    lamc_t = constp.tile([P, HPC], F32, tag="lamc")
    nc.sync.dma_start(lamc_t, lamc_d.ap().rearrange("h p -> p h"))
    mask_t = constp.tile([P, HPC, 2, CHUNK], F16, tag="mask")
    nc.sync.dma_start(mask_t, mask_d.ap().rearrange("h j p c -> p h j c"))
    qdec_t = constp.tile([P, HPC, CHUNK], F16, tag="qdec")
    nc.sync.dma_start(qdec_t, qdec_d.ap().rearrange("h p c -> p h c"))

    # ================= v projection: 4 column-quarters =================
    def prod_v(q, wv_t=None):
        _set_label(f"v{q}")
        if wv_t is None:
            wv_t = wtile(KC, 256)
            nc.sync.dma_start(wv_t, wv_src[:, :, q * 256:(q + 1) * 256])
        for mt in range(T // P):
            psf = mmps.tile([P, 512], F32, tag="mm")
            ps = psf[:, 0:256]
            for kc in range(KC):
                nc.tensor.matmul(ps, xts[kc][:, mt * P:(mt + 1) * P],
                                 wv_t[:, kc],
                                 start=(kc == 0), stop=(kc == KC - 1))
            so = p1so.tile([P, 256], F16, tag="so")
            copy_out(so, ps)
            nc.sync.dma_start(vnds[2 * q][:, mt], so[:, 0:E])
            nc.sync.dma_start(vnds[2 * q + 1][:, mt], so[:, E:2 * E])

    # ================= per-head streaming prefetch =================
    pf_w, pf_cs, pf_v = {}, {}, {}

    def prefetch_w(h):
        _set_label(f"pfw{h}")
        wqk = wtile(KC, 256)
        nc.sync.dma_start(wqk, wt_src[:, :, h * 256:(h + 1) * 256])
        pf_w[h] = wqk

    def prefetch_cs(h):
        _set_label(f"pfcs{h}")
        cost = csstr.tile([P, T], F16, tag="cos")
        sint = csstr.tile([P, T], F16, tag="sin")
        r = h * P
        nc.sync.dma_start(cost, cos_d.ap()[r:r + P])
        nc.sync.dma_start(sint, sin_d.ap()[r:r + P])
        pf_cs[h] = (cost, sint)

    def prefetch_v(h):
        _set_label(f"pfv{h}")
        vna = vstr.tile([P, KC, E], F16, tag="vna")
        nc.sync.dma_start(vna, vnds[h])
        pf_v[h] = vna

    # ================= qk producer (8 substeps: q0 k0 q1 k1 ...) =========
    def prod_qk(h):
        wqk = pf_w[h]
        qs = qkp.tile([P, T], F16, tag="qs")
        ks = qkp.tile([P, T], F16, tag="ks")
        qk_slots[h] = (qs, ks)
        pending = None
        for n in range(NT):
            for which, dst in ((0, qs), (1, ks)):
                _set_label(f"s{h}.mm{n}{'qk'[which]}")
                ps = mmps.tile([P, 512], F32, tag="mm")
                for kc in range(KC):
                    nc.tensor.matmul(ps, wqk[:, kc, which * P:(which + 1) * P],
                                     xts[kc][:, n * 512:(n + 1) * 512],
                                     start=(kc == 0), stop=(kc == KC - 1))
                # the copy-out is emitted AFTER the driver's attention pulls
                # for this substep: a copy whose PSUM isn't complete yet
                # would otherwise head-of-line block ready LRPE work queued
                # behind it on the same engine.
                yield
                _set_label(f"s{h}.cp{n}{'qk'[which]}")
                copy_out(dst[:, n * 512:(n + 1) * 512], ps)

    qk_slots = {}

    # ================= attention consumer =================
    def attn(h):
        """Yields 8 times; yield i emits B(c_{i-1}) then A(c_i), with B(c7)
        emitted on the final drain. A: LRPE + scores + mask + transposes;
        B: output + state update. The B-before-A pairing keeps one full
        producer substep of pipeline lead between a chunk's A-phase
        (cross-engine producers) and its B-phase (PE consumers)."""
        qs, ks = qk_slots[h]
        cost, sint = pf_cs[h]
        vna = pf_v[h]
        lam_col = lamc_t[:, h:h + 1]

        st_prev = [None, None]
        ab = [None] * NCH

        def phase_a(i):
            _set_label(f"a{h}.A{i}")
            sl = slice(i * CHUNK, (i + 1) * CHUNK)
            qlc = p2c.tile([P, CHUNK], F16, tag="qlc")
            qls = p2c.tile([P, CHUNK], F16, tag="qls")
            klc = p2c.tile([P, CHUNK], F16, tag="klc")
            kls = p2c.tile([P, CHUNK], F16, tag="kls")
            nc.vector.tensor_tensor(klc, ks[:, sl], cost[:, sl], mult)
            nc.vector.tensor_tensor(qlc, qs[:, sl], cost[:, sl], mult)
            nc.vector.tensor_tensor(kls, ks[:, sl], sint[:, sl], mult)
            nc.vector.tensor_tensor(qls, qs[:, sl], sint[:, sl], mult)
            # scoresT (both j-halves in one bank) then mask multiply.
            # j-half 1 vs i-half 0 is strictly-upper (future tokens, mask
            # zero), so j-half 1 only computes the i >= 128 half.
            sco = p2ps.tile([P, 2, CHUNK], F32, tag="sco", bufs=2)
            nc.tensor.matmul(sco[:, 0], klc[:, 0:P], qlc,
                             start=True, stop=False)
            nc.tensor.matmul(sco[:, 0], kls[:, 0:P], qls,
                             start=False, stop=True)
            nc.tensor.matmul(sco[:, 1, P:CHUNK], klc[:, P:CHUNK],
                             qlc[:, P:CHUNK], start=True, stop=False)
            nc.tensor.matmul(sco[:, 1, P:CHUNK], kls[:, P:CHUNK],
                             qls[:, P:CHUNK], start=False, stop=True)
            smk = p2c.tile([P, 2, CHUNK], F16, tag="smk")
            nc.vector.tensor_tensor(smk[:, 0], sco[:, 0], mask_t[:, h, 0], mult)
            nc.vector.tensor_tensor(smk[:, 1, P:CHUNK], sco[:, 1, P:CHUNK],
                                    mask_t[:, h, 1, P:CHUNK], mult)
            # k natural layout via PE transpose (in the PE stream -- no
            # cross-engine launch chain) + k-decay fold on the Act copy out
            # of PSUM. Only consumed by the B-phase state update.
            knat = None
            if i < NCH - 1:
                knat = p2c.tile([P, 2, 2 * E], F16, tag="knat")
                ptr = p2ps.tile([P, 2, 2 * E], F16, tag="ptr", bufs=2)
                for half in range(2):
                    hsl = slice(half * P, (half + 1) * P)
                    kd = kdec_t[:, h, half:half + 1]
                    nc.tensor.transpose(ptr[:, half, 0:E], klc[:, hsl], ident16)
                    nc.tensor.transpose(ptr[:, half, E:2 * E], kls[:, hsl], ident16)
                    nc.scalar.activation(knat[:, half, 0:E], ptr[:, half, 0:E],
                                         COPY, bias=0.0, scale=kd)
                    nc.scalar.activation(knat[:, half, E:2 * E],
                                         ptr[:, half, E:2 * E],
                                         COPY, bias=0.0, scale=kd)
            ab[i] = (qlc, qls, knat, None, smk)

        def phase_b(i):
            _set_label(f"a{h}.B{i}")
            nonlocal st_prev
            sl = slice(i * CHUNK, (i + 1) * CHUNK)
            qlc, qls, knat, vnak, smk = ab[i]
            ops = p2ps.tile([E, CHUNK], F32, tag="ops", bufs=1)
            nc.tensor.matmul(ops, vna[:, 2 * i], smk[:, 0],
                             start=True, stop=False)
            nc.tensor.matmul(ops[:, P:CHUNK], vna[:, 2 * i + 1],
                             smk[:, 1, P:CHUNK],
                             start=False, stop=(i == 0), skip_group_check=True)
            if i > 0:
                qsc = p2c.tile([P, CHUNK], F16, tag="qsc")
                qss = p2c.tile([P, CHUNK], F16, tag="qss")
                nc.gpsimd.tensor_tensor(qsc, qlc, qdec_t[:, h], mult)
                nc.gpsimd.tensor_tensor(qss, qls, qdec_t[:, h], mult)
                nc.tensor.matmul(ops, st_prev[0], qsc, start=False, stop=False)
                nc.tensor.matmul(ops, st_prev[1], qss, start=False, stop=True)
            yst = p2c.tile([E, CHUNK], F16, tag="yst")
            nc.scalar.copy(yst, ops)
            half, io = divmod(i, NCH // 2)
            nc.scalar.dma_start(ylTs[half][:, 2 * io:2 * io + 2, h], yst)
            if i < NCH - 1:
                sad = p2ps.tile([P, 2, E], F32, tag="sad", bufs=1)
                nc.tensor.matmul(sad[:, 0], knat[:, 0, 0:E], vna[:, 2 * i],
                                 start=True, stop=False)
                nc.tensor.matmul(sad[:, 0], knat[:, 1, 0:E], vna[:, 2 * i + 1],
                                 start=False, stop=True)
                nc.tensor.matmul(sad[:, 1], knat[:, 0, E:2 * E], vna[:, 2 * i],
                                 start=True, stop=False)
                nc.tensor.matmul(sad[:, 1], knat[:, 1, E:2 * E],
                                 vna[:, 2 * i + 1],
                                 start=False, stop=True)
                st0 = stp.tile([P, E], F16, tag="st0")
                st1 = stp.tile([P, E], F16, tag="st1")
                if i == 0:
                    nc.vector.tensor_copy(out=st0, in_=sad[:, 0])
                    nc.vector.tensor_copy(out=st1, in_=sad[:, 1])
                else:
                    nc.vector.scalar_tensor_tensor(
                        out=st0, in0=st_prev[0], scalar=lam_col,
                        in1=sad[:, 0], op0=mult, op1=add)
                    nc.vector.scalar_tensor_tensor(
                        out=st1, in0=st_prev[1], scalar=lam_col,
                        in1=sad[:, 1], op0=mult, op1=add)
                st_prev = [st0, st1]

        for i in range(NCH):
            if i > 0:
                phase_b(i - 1)
            phase_a(i)
            yield
        phase_b(NCH - 1)

    # ================= drive =================
    prod_v(0, wv_t0)
    for q in range(1, 4):
        prod_v(q)
    prefetch_v(0)
    prefetch_v(1)
    prefetch_cs(0)
    prefetch_cs(1)
    prefetch_w(0)     # ring slot of vq2 -- free once vq2's matmuls retire

    # Self-zipped pipeline with a floating consumer queue: after each
    # producer substep, pull one [B(c_{i-1}), A(c_i)] attention step from
    # the oldest unfinished head. Heads have 8 steps and slots 8 substeps,
    # so in steady state head h drains across the second half of slot h and
    # the first half of slot h+1 -- every phase keeps >= 1 substep of
    # pipeline lead for its copy->LRPE->transpose chain and there is no
    # per-head tail bunching.
    consq = []

    def pull(n):
        while n > 0 and consq:
            try:
                next(consq[0])
                n -= 1
            except StopIteration:
                consq.pop(0)

    for h in range(HPC):
        prod = prod_qk(h)
        for j in range(8):
            next(prod)
            if j == 1 and h + 1 < HPC:
                prefetch_w(h + 1)
            if j == 2:
                consq.append(attn(h))
            if j == 3 and h + 2 < HPC:
                prefetch_cs(h + 2)
                prefetch_v(h + 2)
            pull(1)
        for _ in prod:
            pass
    # wp into the (now dead) xts tiles: xts[kc] <- w_proj rows kc*128..+128.
    # The WAR on the last qk pair's reads orders these after pair 7.
    _set_label("p3.wp")
    for kc in range(HPC):
        nc.sync.dma_start(xts[kc], wp_src[:, kc])
    while consq:
        pull(1)

    # ================= output projection (m-outer, yl read once) =========
    for m in range(T // P):
        _set_label(f"p3.m{m}")
        yl = p3y.tile([P, HPC, P], F16, tag="yl")
        nc.sync.dma_start(yl, ylTs[m // 8][:, m % 8])
        for n in range(NT):
            ps = mmps.tile([P, 512], F32, tag="mm")
            for kc in range(HPC):
                nc.tensor.matmul(ps, yl[:, kc],
                                 xts[kc][:, n * 512:(n + 1) * 512],
                                 start=(kc == 0), stop=(kc == HPC - 1))
            so = p3so.tile([P, 512], F16, tag="p3o")
            copy_out(so, ps)
            nc.sync.dma_start(
                out_d.ap()[m * P:(m + 1) * P, n * 512:(n + 1) * 512], so)


def _get_nc():
    global _NC_CACHE
    if _NC_CACHE is None:
        _NC_CACHE = _build_nc()
    return _NC_CACHE


def _slopes(h):
    start = 2.0 ** (-(2.0 ** -(math.log2(h) - 3)))
    return np.array([start ** (i + 1) for i in range(h)], dtype=np.float64)


def _prepare_in_maps(x, w_qkv, w_proj, theta):
    slopes = _slopes(HEADS)
    t = np.arange(T, dtype=np.float64)
    idx = np.arange(CHUNK, dtype=np.float64)

    in_maps = []
    for core in range(8):
        b, g = divmod(core, 2)
        heads = np.arange(g * HPC, (g + 1) * HPC)

        xt = np.ascontiguousarray(x[b].T).astype(np.float16)

        # columns pair-interleaved: head h -> [q rows | k rows]
        qk_rows = np.concatenate([
            np.concatenate([
                np.arange(g * YD + hh * E, g * YD + (hh + 1) * E),
                np.arange(DIM + g * YD + hh * E, DIM + g * YD + (hh + 1) * E),
            ]) for hh in range(HPC)
        ])
        wt = np.ascontiguousarray(w_qkv[qk_rows].T).astype(np.float16)
        v_rows = np.arange(2 * DIM + g * YD, 2 * DIM + (g + 1) * YD)
        wv = np.ascontiguousarray(w_qkv[v_rows].T).astype(np.float16)

        wp = np.ascontiguousarray(w_proj[:, g * YD:(g + 1) * YD].T).astype(np.float16)

        th = theta.reshape(HEADS, E)[heads].astype(np.float64)  # (8, 128)
        ang = th[:, :, None] * t[None, None, :]                 # (8, 128, T)
        costab = np.cos(ang).astype(np.float16).reshape(YD, T)
        sintab = np.sin(ang).astype(np.float16).reshape(YD, T)

        s = slopes[heads]                                       # (8,)
        diff = idx[:, None] - idx[None, :]                      # (i, j)
        maskt = np.where(
            diff[None] >= 0, np.exp(-s[:, None, None] * diff[None]), 0.0
        )                                                       # (8, i, j) = diag_decay
        maskt = np.ascontiguousarray(
            maskt.transpose(0, 2, 1).reshape(HPC, 2, P, CHUNK)).astype(np.float16)
        qdec = np.exp(-s[:, None] * (idx + 1.0)[None]).astype(np.float16)  # (8, 256)
        qdec = np.broadcast_to(qdec[:, None, :], (HPC, P, CHUNK)).copy()
        kdec = np.exp(-s[:, None] * (CHUNK - 1.0 - idx)[None]).astype(np.float32)
        kdec = np.ascontiguousarray(kdec.reshape(HPC, 2, P))
        lamc = np.exp(-s * CHUNK).astype(np.float32)            # (8,)
        lamc = np.broadcast_to(lamc[:, None], (HPC, P)).copy()

        in_maps.append({
            "xt": xt, "wt": wt, "wv": wv, "wp": wp,
            "costab": costab, "sintab": sintab,
            "maskt": maskt, "qdec": qdec, "kdec": kdec, "lamc": lamc,
        })
    return in_maps


def kernel(x, w_qkv, w_proj, theta):
    x = np.asarray(x)
    w_qkv = np.asarray(w_qkv)
    w_proj = np.asarray(w_proj)
    theta = np.asarray(theta)

    nc = _get_nc()
    in_maps = _prepare_in_maps(x, w_qkv, w_proj, theta)
    res = bass_utils.run_bass_kernel_spmd(nc, in_maps, core_ids=list(range(8)))

    out = np.empty((B, T, DIM), dtype=np.float32)
    for b in range(B):
        out[b] = (res.results[2 * b]["out"].astype(np.float32)
                  + res.results[2 * b + 1]["out"].astype(np.float32))
    return out
